# revision 1
# baseline (speedup 1.0000x reference)
"""Llama attention layer (B=2, S=2048, H=4096, 32 heads, fp32 io) on 8 trn2 cores.

Sharding: tensor-parallel over heads. Each core owns 4 heads: W_qkv column
shard [4096, 3*512] (bf16), W_o row shard [512, 4096] (bf16). Each core
computes qkv proj + RoPE + causal attention for its heads + its o_proj
partial; the host sums the 8 fp32 partials (the "all-reduce").

Device kernel (per core), all matmuls bf16 with fp32 PSUM accumulation:
  phase 1: q/k/v = hiddenT-tiles.T @ W-chunks in natural [token, feat]
           layout; RoPE applied with free-dim slices; results bounced to
           DRAM scratch.
  phase 2: per (batch, head): load Q^T/K^T via DMA-transpose, V natural.
           S^T[k,q] = K^T-tile.T @ Q^T ; P = exp(SCALING*S) (no max-sub:
           |scores|<~6 so exp is safe in fp32) ; causal handled by
           multiplicative masks on 4 diagonal block offsets ; attn^T =
           V.T @ P^T accumulated over k-chunks; row sums l via ones-matmul;
           normalize by broadcast reciprocal.
  phase 3: o_partial[t, :] = attn^T-tiles.T @ W_o shard, accumulated over
           the 4 head-chunks, written as fp32.
"""

import numpy as np
import ml_dtypes

import concourse.bass as bass
import concourse.tile as tile
from concourse import bacc, mybir
from concourse.bass_utils import run_bass_kernel_spmd

# ---- problem constants (hardcoded per contract) ----
HIDDEN = 4096
NH = 32
D = 128
B = 2
S = 2048
TOK = B * S            # 4096 tokens
N_CORES = 8
HC = NH // N_CORES     # 4 heads per core
FH = HC * D            # 512 features per core for each of q/k/v
SCALING = D ** -0.5
ROPE_BASE = 10000.0

BF16 = mybir.dt.bfloat16
F32 = mybir.dt.float32

TBLK = 512             # tokens per phase-1 block
NTB = TOK // TBLK      # 8
HKC = 16               # h-dim 128-chunks per hT/w chunk tile (2 chunks = full 4096)
QT = 512               # q columns per phase-2 tile
NQT = S // QT          # 4
NKC = S // 128         # 16 k chunks per sequence


def _emit_phase1_block(nc, T, pools, dram):
    """One 512-token block: q/k/v projections (natural layout) + rope."""
    hp, wp, csp, rtp, stp, psp = (
        pools["hblk"], pools["wch"], pools["cs"], pools["rtmp"],
        pools["stage"], pools["ps"],
    )
    hT, w, csn = dram["hT"], dram["w"], dram["csn"]
    scr = [dram["q_nat"], dram["k_nat"], dram["v_nat"]]

    # hT block: [4096 h, 512 t] as 4 chunks of [128 p, 8 kc, 512 t]
    hblk = []
    for i in range(4):
        t_ = hp.tile([128, 8, TBLK], BF16, tag="hblk")
        nc.sync.dma_start(
            out=t_,
            in_=hT[i * 1024:(i + 1) * 1024, T * TBLK:(T + 1) * TBLK].rearrange(
                "(kc p) t -> p kc t", p=128),
        )
        hblk.append(t_)

    csts = []

    for j3 in range(3):  # q, k, v
        # w chunk: [4096 h, 512 f] as 4 chunks of [128 p, 8 kc, 512 f]
        wch = []
        for i in range(4):
            t_ = wp.tile([128, 8, FH], BF16, tag="wch")
            nc.sync.dma_start(
                out=t_,
                in_=w[i * 1024:(i + 1) * 1024, j3 * FH:(j3 + 1) * FH].rearrange(
                    "(kc p) f -> p kc f", p=128),
            )
            wch.append(t_)

        if j3 == 0:
            # cos/sin tiles per token subtile: [128 t, 2, 4, 128]
            for tt in range(4):
                cst = csp.tile([128, 2, HC, D], BF16, tag="cs")
                r0 = T * TBLK + tt * 128
                nc.sync.dma_start(
                    out=cst,
                    in_=csn[r0:r0 + 128, :, :].rearrange(
                        "p c (h d) -> p c h d", h=HC),
                )
                csts.append(cst)

        for tt in range(4):
            ps = psp.tile([128, HC, D], F32, tag="ps")
            for i in range(4):
                for kc in range(8):
                    nc.tensor.matmul(
                        ps,
                        lhsT=hblk[i][:, kc, tt * 128:(tt + 1) * 128],
                        rhs=wch[i][:, kc, :],
                        start=(i == 0 and kc == 0),
                        stop=(i == 3 and kc == 7),
                    )
            st = stp.tile([128, HC, D], BF16, tag="stage")
            if j3 < 2:
                cst = csts[tt]
                half = D // 2
                tr = rtp.tile([128, HC, D], F32, tag="rtmp")
                tcos = rtp.tile([128, HC, D], F32, tag="rtmp")
                # rotate-half times signed sin
                nc.vector.tensor_mul(tr[:, :, 0:half], ps[:, :, half:D], cst[:, 1, :, 0:half])
                nc.vector.tensor_mul(tr[:, :, half:D], ps[:, :, 0:half], cst[:, 1, :, half:D])
                nc.vector.tensor_mul(tcos, ps, cst[:, 0])
                nc.vector.tensor_add(st, tr, tcos)
            else:
                nc.vector.tensor_copy(out=st, in_=ps)
            r0 = T * TBLK + tt * 128
            nc.sync.dma_start(out=scr[j3][r0:r0 + 128, :], in_=st)


def _emit_phase2_pair(nc, b, hh, pools, dram, attn_s, mskt, ones_col,
                      qt_hook=None):
    """Causal attention for one (batch, local-head) pair.

    k-chunks processed in groups of 2 so one ACTIVATE covers [128, 2*QT]
    (amortizes the ~400ns per-instruction ACT overhead)."""
    pairp, ptp, pvlp, ps2p, smallp = (
        pools["pair"], pools["pt"], pools["pvl"], pools["ps2"], pools["small"],
    )
    q_nat, k_nat, v_nat = dram["q_nat"], dram["k_nat"], dram["v_nat"]
    rows = slice(b * S, (b + 1) * S)
    cols = slice(hh * D, (hh + 1) * D)

    qTp = pairp.tile([128, S], BF16, tag="pair")
    nc.sync.dma_start_transpose(out=qTp, in_=q_nat[rows, cols])
    kTp = pairp.tile([128, S], BF16, tag="pair")
    nc.sync.dma_start_transpose(out=kTp, in_=k_nat[rows, cols])
    vp = pairp.tile([128, NKC, D], BF16, tag="pair")
    nc.sync.dma_start(
        out=vp, in_=v_nat[rows, cols].rearrange("(kc p) d -> p kc d", p=128))

    for qt in range(NQT):
        qcols = slice(qt * QT, (qt + 1) * QT)
        pv = pvlp.tile([128, QT], F32, tag="pvl")
        pl = pvlp.tile([128, QT], F32, tag="pvl", name="pl")
        nkc = 4 * (qt + 1)
        for g in range(nkc // 2):
            sc = ps2p.tile([128, 2, QT], F32, tag="ps2")
            for s2 in range(2):
                kc = 2 * g + s2
                nc.tensor.matmul(
                    sc[:, s2, :], lhsT=kTp[:, kc * 128:(kc + 1) * 128],
                    rhs=qTp[:, qcols], start=True, stop=True)
            pt2 = ptp.tile([128, 2, QT], BF16, tag="pt")
            nc.scalar.activation(
                out=pt2, in_=sc, func=mybir.ActivationFunctionType.Exp,
                scale=SCALING)
            if g >= 2 * qt:  # diagonal group: apply causal mask pair
                pt2m = ptp.tile([128, 2, QT], BF16, tag="pt")
                nc.vector.tensor_mul(pt2m, pt2, mskt[:, g - 2 * qt])
                pt2 = pt2m
            for s2 in range(2):
                kc = 2 * g + s2
                nc.tensor.matmul(
                    pv, lhsT=vp[:, kc, :], rhs=pt2[:, s2, :],
                    start=(kc == 0), stop=(kc == nkc - 1))
                nc.tensor.matmul(
                    pl[0:1, :], lhsT=ones_col, rhs=pt2[:, s2, :],
                    start=(kc == 0), stop=(kc == nkc - 1))
        # normalize: attn^T[:, qcols] = pv * (1/l) broadcast over partitions
        linv = smallp.tile([1, QT], F32, tag="linv1")
        nc.vector.reciprocal_approx_fast(out=linv, in_=pl[0:1, :])
        linv_bc = smallp.tile([128, QT], F32, tag="linvbc")
        nc.gpsimd.partition_broadcast(linv_bc, linv)
        nc.vector.tensor_mul(
            attn_s[b][:, hh, qt * QT:(qt + 1) * QT], pv, linv_bc)
        if qt_hook is not None:
            qt_hook(qt)


def _emit_phase3_wo(nc, pools, dram):
    wot = pools["wo"].tile([128, HC, HIDDEN], BF16, tag="wo")
    nc.sync.dma_start(out=wot, in_=dram["wo"].rearrange("(kc p) o -> p kc o", p=128))
    return wot


def _emit_phase3_sts(nc, pools, dram, attn_s, wot, sts):
    """o_proj partial for a range of 128-token tiles."""
    ostp, psp = pools["ost"], pools["ps"]
    outp = dram["outp"]

    for st in sts:
        for ocp in range(4):  # pairs of 512-col output tiles
            pso = [psp.tile([128, QT], F32, tag="ps", name="pso0"),
                   psp.tile([128, QT], F32, tag="ps", name="pso1")]
            for kc in range(HC):
                for oc in range(2):
                    o0 = (ocp * 2 + oc) * QT
                    nc.tensor.matmul(
                        pso[oc],
                        lhsT=attn_s[st // 16][:, kc, (st % 16) * 128:
                                              (st % 16 + 1) * 128],
                        rhs=wot[:, kc, o0:o0 + QT],
                        start=(kc == 0), stop=(kc == HC - 1))
            for oc in range(2):
                o0 = (ocp * 2 + oc) * QT
                ot = ostp.tile([128, QT], F32, tag="ost")
                if oc == 0:
                    nc.vector.tensor_copy(out=ot, in_=pso[oc])
                else:
                    nc.scalar.copy(out=ot, in_=pso[oc])
                nc.sync.dma_start(
                    out=outp[st * 128:(st + 1) * 128, o0:o0 + QT], in_=ot)


def build_nc():
    nc = bacc.Bacc("TRN2", target_bir_lowering=False, debug=False,
                   num_devices=N_CORES)
    dram = {
        "hT": nc.dram_tensor("hT", [HIDDEN, TOK], BF16, kind="ExternalInput").ap(),
        "w": nc.dram_tensor("w", [HIDDEN, 3 * FH], BF16, kind="ExternalInput").ap(),
        "wo": nc.dram_tensor("wo", [FH, HIDDEN], BF16, kind="ExternalInput").ap(),
        "csn": nc.dram_tensor("csn", [TOK, 2, FH], BF16, kind="ExternalInput").ap(),
        "msk": nc.dram_tensor("msk", [128, 2, 2, QT], BF16, kind="ExternalInput").ap(),
        "outp": nc.dram_tensor("outp", [TOK, HIDDEN], F32, kind="ExternalOutput").ap(),
    }

    with tile.TileContext(nc) as tc:
        from contextlib import ExitStack
        with ExitStack() as ctx:
            # global pools (live across phases)
            pools = {}
            pools["ps"] = ctx.enter_context(tc.tile_pool(name="ps", bufs=4, space="PSUM"))
            pools["pvl"] = ctx.enter_context(tc.tile_pool(name="pvl", bufs=2, space="PSUM"))
            pools["ps2"] = ctx.enter_context(tc.tile_pool(name="ps2", bufs=1, space="PSUM"))
            gsb = ctx.enter_context(tc.tile_pool(name="gsb", bufs=1))
            pools["pair"] = ctx.enter_context(tc.tile_pool(name="pair", bufs=6))
            pools["pt"] = ctx.enter_context(tc.tile_pool(name="pt", bufs=6))
            pools["small"] = ctx.enter_context(tc.tile_pool(name="small", bufs=2))
            dscr = ctx.enter_context(tc.tile_pool(name="dscr", bufs=1, space="DRAM"))

            dram["q_nat"] = dscr.tile([TOK, FH], BF16, tag="qs", name="q_nat")
            dram["k_nat"] = dscr.tile([TOK, FH], BF16, tag="ks", name="k_nat")
            dram["v_nat"] = dscr.tile([TOK, FH], BF16, tag="vs", name="v_nat")

            attn_b0 = gsb.tile([128, HC, S], BF16, tag="attn0")
            attn_b1 = gsb.tile([128, HC, S], BF16, tag="attn1")
            attn_s = [attn_b0, attn_b1]
            mskt = gsb.tile([128, 2, 2, QT], BF16, tag="msk")
            ones_col = gsb.tile([128, 1], BF16, tag="ones_col")

            with ExitStack() as p1ctx:
                pools["hblk"] = p1ctx.enter_context(tc.tile_pool(name="hblk", bufs=6))
                pools["wch"] = p1ctx.enter_context(tc.tile_pool(name="wch", bufs=6))
                pools["cs"] = p1ctx.enter_context(tc.tile_pool(name="cs", bufs=5))
                pools["rtmp"] = p1ctx.enter_context(tc.tile_pool(name="rtmp", bufs=3))
                pools["stage"] = p1ctx.enter_context(tc.tile_pool(name="stage", bufs=6))

                # b0 blocks; then interleave b0 attention with b1 blocks
                for T in range(NTB // 2):
                    _emit_phase1_block(nc, T, pools, dram)
                nc.sync.dma_start(out=mskt, in_=dram["msk"])
                nc.vector.memset(ones_col, 1.0)
                for hh in range(HC):
                    _emit_phase2_pair(nc, 0, hh, pools, dram, attn_s, mskt,
                                      ones_col)
                    _emit_phase1_block(nc, NTB // 2 + hh, pools, dram)

            with ExitStack() as p3ctx:
                pools["wo"] = p3ctx.enter_context(tc.tile_pool(name="wo", bufs=1))
                pools["ost"] = p3ctx.enter_context(tc.tile_pool(name="ost", bufs=4))
                wot = _emit_phase3_wo(nc, pools, dram)
                # interleave b1 attention with b0-token o_proj
                for hh in range(HC):
                    _emit_phase2_pair(nc, 1, hh, pools, dram, attn_s, mskt,
                                      ones_col)
                    _emit_phase3_sts(nc, pools, dram, attn_s, wot,
                                     range(4 * hh, 4 * hh + 4))
                _emit_phase3_sts(nc, pools, dram, attn_s, wot, range(16, 32))

    nc.compile()
    return nc


_NC_CACHE = {}


def get_nc():
    if "nc" not in _NC_CACHE:
        _NC_CACHE["nc"] = build_nc()
    return _NC_CACHE["nc"]


def prep_in_maps(positions, hidden_states, W_qkv, W_o):
    """Host-side sharding + layout prep. Returns per-core input maps."""
    bf16 = ml_dtypes.bfloat16
    hid = np.asarray(hidden_states, np.float32).reshape(TOK, HIDDEN)
    hT = np.ascontiguousarray(hid.T).astype(bf16)

    pos = np.asarray(positions).reshape(TOK).astype(np.float32)
    half = D // 2
    inv = ROPE_BASE ** (-np.arange(half, dtype=np.float32) / half)
    ang = pos[:, None] * inv[None, :]                      # [TOK, 64]
    cos = np.cos(ang)
    sin = np.sin(ang)
    cos128 = np.concatenate([cos, cos], axis=1)            # [TOK, 128]
    sin128 = np.concatenate([-sin, sin], axis=1)
    csn = np.empty((TOK, 2, FH), np.float32)
    csn[:, 0, :] = np.tile(cos128, HC)
    csn[:, 1, :] = np.tile(sin128, HC)
    csn = csn.astype(bf16)

    kk = np.arange(128)[:, None]
    qq = np.arange(QT)[None, :]
    msk = np.stack([(qq >= kk + o * 128) for o in range(4)], axis=1)
    msk = msk.reshape(128, 2, 2, QT).astype(bf16)           # [128, 2, 2, 512]

    Wq = np.asarray(W_qkv, np.float32)
    Wo = np.asarray(W_o, np.float32)
    in_maps = []
    for c in range(N_CORES):
        wc = np.concatenate(
            [Wq[:, q0 * HIDDEN + c * FH: q0 * HIDDEN + (c + 1) * FH]
             for q0 in range(3)], axis=1).astype(bf16)
        woc = np.ascontiguousarray(Wo[c * FH:(c + 1) * FH, :]).astype(bf16)
        in_maps.append({"hT": hT, "w": wc, "wo": woc, "csn": csn, "msk": msk})
    return in_maps


def kernel(positions, hidden_states, W_qkv, W_o):
    nc = get_nc()
    in_maps = prep_in_maps(positions, hidden_states, W_qkv, W_o)
    res = run_bass_kernel_spmd(nc, in_maps, list(range(N_CORES)))
    out = res.results[0]["outp"].astype(np.float64)
    for c in range(1, N_CORES):
        out += res.results[c]["outp"]
    return out.astype(np.float32).reshape(B, S, HIDDEN)



# revision 2
# speedup vs baseline: 1.0061x; 1.0061x over previous
"""Llama attention layer (B=2, S=2048, H=4096, 32 heads, fp32 io) on 8 trn2 cores.

Sharding: tensor-parallel over heads. Each core owns 4 heads: W_qkv column
shard [4096, 3*512] (bf16), W_o row shard [512, 4096] (bf16). Each core
computes qkv proj + RoPE + causal attention for its heads + its o_proj
partial; the host sums the 8 fp32 partials (the "all-reduce") and
untransposes the output (kernel emits o_partial^T).

v2 vs baseline (tensor-engine work reduction + stall elimination):
  - softmax row-sums no longer use 128*1*512 matmuls (94us of PE time);
    P tiles are accumulated on Vector and partition-summed on GpSimd.
  - diagonal causal blocks stream only the unmasked column range in the
    scores and PV matmuls (exp trimmed to match).
  - phase 3 is weight-stationary: out^T[o,t] = Wo_chunk^T @ attn^T tiles,
    so only a [128,4,128] Wo tile is resident at a time.
  - emission-level scheduling: phase-1 / phase-3 work is emitted in small
    units between phase-2 groups (lag-1 pipelined) so the PE never idles
    on the exp-activation chain and stays at full p-state.
  - hT / W_qkv / W_o are pre-arranged on host so every DMA line is one
    contiguous 1-8KB descriptor per partition.
"""

import numpy as np
import ml_dtypes

import concourse.bass as bass
import concourse.tile as tile
from concourse import bacc, mybir
from concourse.bass_isa import ReduceOp
from concourse.bass_utils import run_bass_kernel_spmd

# ---- problem constants (hardcoded per contract) ----
HIDDEN = 4096
NH = 32
D = 128
B = 2
S = 2048
TOK = B * S            # 4096 tokens
N_CORES = 8
HC = NH // N_CORES     # 4 heads per core
FH = HC * D            # 512 features per core for each of q/k/v
SCALING = D ** -0.5
ROPE_BASE = 10000.0

BF16 = mybir.dt.bfloat16
F32 = mybir.dt.float32

TBLK = 512             # tokens per phase-1 block
NTB = TOK // TBLK      # 8
QT = 512               # q columns per phase-2 tile
NQT = S // QT          # 4
NKC = S // 128         # 16 k chunks per sequence
NOB = HIDDEN // 128    # 32 output-column chunks in phase 3
EXP = mybir.ActivationFunctionType.Exp


class Filler:
    """Pulls emission units (generators yielding ~tensor-ns) on demand."""

    def __init__(self, gens):
        self.gens = list(gens)

    def pull(self, ns):
        while ns > 0 and self.gens:
            try:
                ns -= next(self.gens[0])
            except StopIteration:
                self.gens.pop(0)

    def drain(self):
        self.pull(float("inf"))


def build_nc():
    nc = bacc.Bacc("TRN2", target_bir_lowering=False, debug=False,
                   num_devices=N_CORES)
    hT = nc.dram_tensor("hT", [NTB, 4, 128, 8 * TBLK], BF16, kind="ExternalInput").ap()
    w = nc.dram_tensor("w", [3, 4, 128, 8 * FH], BF16, kind="ExternalInput").ap()
    wo = nc.dram_tensor("wo", [NOB, 128, HC, 128], BF16, kind="ExternalInput").ap()
    csn = nc.dram_tensor("csn", [TOK, 2, FH], BF16, kind="ExternalInput").ap()
    msk = nc.dram_tensor("msk", [128, 2, 2, QT], BF16, kind="ExternalInput").ap()
    outp = nc.dram_tensor("outp", [HIDDEN, TOK], F32, kind="ExternalOutput").ap()

    with tile.TileContext(nc) as tc:
        from contextlib import ExitStack
        with ExitStack() as ctx:
            # PSUM: ps 2 banks + ps2 4 banks + pv 2 banks = 8
            psp = ctx.enter_context(tc.tile_pool(name="ps", bufs=2, space="PSUM"))
            ps2p = ctx.enter_context(tc.tile_pool(name="ps2", bufs=2, space="PSUM"))
            pvp = ctx.enter_context(tc.tile_pool(name="pv", bufs=2, space="PSUM"))
            gsb = ctx.enter_context(tc.tile_pool(name="gsb", bufs=1))
            pairp = ctx.enter_context(tc.tile_pool(name="pair", bufs=6))
            ptp = ctx.enter_context(tc.tile_pool(name="pt", bufs=6))
            accp = ctx.enter_context(tc.tile_pool(name="acc", bufs=2))
            lnvp = ctx.enter_context(tc.tile_pool(name="lnv", bufs=2))
            dscr = ctx.enter_context(tc.tile_pool(name="dscr", bufs=1, space="DRAM"))

            q_nat = dscr.tile([TOK, FH], BF16, tag="qs", name="q_nat")
            k_nat = dscr.tile([TOK, FH], BF16, tag="ks", name="k_nat")
            v_nat = dscr.tile([TOK, FH], BF16, tag="vs", name="v_nat")
            scr = [q_nat, k_nat, v_nat]

            attn_s = [gsb.tile([128, HC, S], BF16, tag="attn0", name="attn0"),
                      gsb.tile([128, HC, S], BF16, tag="attn1", name="attn1")]
            mskt = gsb.tile([128, 2, 2, QT], BF16, tag="msk", name="mskt")
            nc.sync.dma_start(out=mskt, in_=msk)

            # ---------------- phase 2: one (batch, head) pair ----------------
            def prefetch_pair(b, hh):
                rows = slice(b * S, (b + 1) * S)
                cols = slice(hh * D, (hh + 1) * D)
                qTp = pairp.tile([128, S], BF16, tag="pair", name="qTp")
                nc.sync.dma_start_transpose(out=qTp, in_=q_nat[rows, cols])
                kTp = pairp.tile([128, S], BF16, tag="pair", name="kTp")
                nc.sync.dma_start_transpose(out=kTp, in_=k_nat[rows, cols])
                vp = pairp.tile([128, NKC, D], BF16, tag="pair", name="vp")
                nc.sync.dma_start(
                    out=vp,
                    in_=v_nat[rows, cols].rearrange("(kc p) d -> p kc d", p=128))
                return qTp, kTp, vp

            def run_pair(b, hh, tiles, filler):
                qTp, kTp, vp = tiles
                for qt in range(NQT):
                    nkc = 4 * (qt + 1)
                    pv = pvp.tile([128, QT], F32, tag="pv", name="pv")
                    acc = accp.tile([128, QT], F32, tag="acc", name="acc")
                    prev_pv = None
                    for g in range(nkc // 2):
                        diag = g >= 2 * qt
                        rs = [max(0, 128 * (2 * g + s2 - 4 * qt)) for s2 in (0, 1)]
                        sc = ps2p.tile([128, 2, QT], F32, tag="ps2", name="sc")
                        for s2 in range(2):
                            kc = 2 * g + s2
                            nc.tensor.matmul(
                                sc[:, s2, rs[s2]:],
                                lhsT=kTp[:, kc * 128:(kc + 1) * 128],
                                rhs=qTp[:, qt * QT + rs[s2]:(qt + 1) * QT],
                                start=True, stop=True)
                        pt2 = ptp.tile([128, 2, QT], BF16, tag="pt", name="pt2")
                        if diag:
                            for s2 in range(2):
                                r = rs[s2]
                                nc.scalar.activation(
                                    out=pt2[:, s2, r:], in_=sc[:, s2, r:],
                                    func=EXP, scale=SCALING)
                            psel = ptp.tile([128, 2, QT], BF16, tag="pt", name="ptm")
                            for s2 in range(2):
                                r = rs[s2]
                                nc.vector.tensor_mul(
                                    psel[:, s2, r:], pt2[:, s2, r:],
                                    mskt[:, g - 2 * qt, s2, r:])
                        else:
                            nc.scalar.activation(out=pt2, in_=sc, func=EXP,
                                                 scale=SCALING)
                            psel = pt2
                        # softmax denominator accumulation (Vector, fp32)
                        if g == 0:
                            if diag:  # qt == 0: rs == [0, 128]
                                nc.vector.tensor_copy(out=acc, in_=psel[:, 0, :])
                                nc.vector.tensor_add(
                                    acc[:, 128:], acc[:, 128:], psel[:, 1, 128:])
                            else:
                                nc.vector.tensor_add(acc, psel[:, 0, :], psel[:, 1, :])
                        else:
                            for s2 in range(2):
                                r = rs[s2]
                                nc.vector.tensor_add(
                                    acc[:, r:], acc[:, r:], psel[:, s2, r:])
                        filler.pull(700)
                        if prev_pv is not None:
                            prev_pv()

                        def mk_pv(psel_, g_, rs_):
                            def emit():
                                for s2 in range(2):
                                    kc = 2 * g_ + s2
                                    nc.tensor.matmul(
                                        pv[:, rs_[s2]:], lhsT=vp[:, kc, :],
                                        rhs=psel_[:, s2, rs_[s2]:],
                                        start=(kc == 0), stop=(kc == nkc - 1))
                            return emit
                        prev_pv = mk_pv(psel, g, rs)
                    prev_pv()
                    # l = column sums of acc; normalize into attn_s
                    nc.gpsimd.partition_all_reduce(acc, acc, 128, ReduceOp.add)
                    linv = lnvp.tile([128, QT], F32, tag="lnv", name="linv")
                    nc.vector.reciprocal_approx_fast(out=linv, in_=acc)
                    nc.vector.tensor_mul(
                        attn_s[b][:, hh, qt * QT:(qt + 1) * QT], pv, linv)

            # ---------------- phase 1: qkv projections + rope ----------------
            def p1_stream(blocks, pools):
                hp, wp, csp, rtp, stp = pools
                for T in blocks:
                    hblk = []
                    for i in range(4):
                        t_ = hp.tile([128, 8, TBLK], BF16, tag="hblk", name="hblk")
                        nc.sync.dma_start(
                            out=t_, in_=hT[T, i].rearrange("p (kc t) -> p kc t", kc=8))
                        hblk.append(t_)
                    csts = []
                    for tt in range(4):
                        cst = csp.tile([128, 2, HC, D], BF16, tag="cs", name="cst")
                        r0 = T * TBLK + tt * 128
                        nc.sync.dma_start(
                            out=cst,
                            in_=csn[r0:r0 + 128].rearrange("p c (h d) -> p c h d", h=HC))
                        csts.append(cst)
                    yield 0
                    for j3 in range(3):
                        wch = []
                        for i in range(4):
                            t_ = wp.tile([128, 8, FH], BF16, tag="wch", name="wch")
                            nc.sync.dma_start(
                                out=t_, in_=w[j3, i].rearrange("p (kc f) -> p kc f", kc=8))
                            wch.append(t_)
                        yield 0
                        for tt in range(4):
                            ps = psp.tile([128, HC, D], F32, tag="ps", name="ps")
                            for half in range(8):
                                i = half // 2
                                for kc in range(4 * (half % 2), 4 * (half % 2) + 4):
                                    nc.tensor.matmul(
                                        ps,
                                        lhsT=hblk[i][:, kc, tt * 128:(tt + 1) * 128],
                                        rhs=wch[i][:, kc, :],
                                        start=(half == 0 and kc == 0),
                                        stop=(half == 7 and kc == 7),
                                    )
                                yield 852
                            st = stp.tile([128, HC, D], BF16, tag="stage", name="st")
                            if j3 < 2:
                                cst = csts[tt]
                                half_d = D // 2
                                tr = rtp.tile([128, HC, D], F32, tag="rtmp", name="tr")
                                tcos = rtp.tile([128, HC, D], F32, tag="rtmp", name="tcos")
                                nc.vector.tensor_mul(
                                    tr[:, :, 0:half_d], ps[:, :, half_d:D],
                                    cst[:, 1, :, 0:half_d])
                                nc.vector.tensor_mul(
                                    tr[:, :, half_d:D], ps[:, :, 0:half_d],
                                    cst[:, 1, :, half_d:D])
                                nc.vector.tensor_mul(tcos, ps, cst[:, 0])
                                nc.vector.tensor_add(st, tr, tcos)
                            else:
                                nc.vector.tensor_copy(out=st, in_=ps)
                            r0 = T * TBLK + tt * 128
                            nc.sync.dma_start(out=scr[j3][r0:r0 + 128, :], in_=st)
                            yield 0

            # ---------------- phase 3: o_proj partial (transposed out) -------
            def p3_stream(tbs, pools):
                wop, ostp = pools
                for ob in range(NOB):
                    wot = wop.tile([128, HC, 128], BF16, tag="wo", name="wot")
                    nc.sync.dma_start(out=wot, in_=wo[ob])
                    yield 0
                    for n, tb in enumerate(tbs):
                        pso = psp.tile([128, TBLK], F32, tag="ps", name="pso")
                        for kc in range(HC):
                            nc.tensor.matmul(
                                pso, lhsT=wot[:, kc, :],
                                rhs=attn_s[tb // 4][:, kc,
                                                    (tb % 4) * TBLK:(tb % 4 + 1) * TBLK],
                                start=(kc == 0), stop=(kc == HC - 1))
                        yield 852
                        ot = ostp.tile([128, TBLK], F32, tag="ost", name="ot")
                        if n % 2 == 0:
                            nc.vector.tensor_copy(out=ot, in_=pso)
                        else:
                            nc.scalar.copy(out=ot, in_=pso)
                        nc.sync.dma_start(
                            out=outp[ob * 128:(ob + 1) * 128,
                                     tb * TBLK:(tb + 1) * TBLK], in_=ot)
                        yield 0

            # ---------------- schedule -------------------------------------
            with ExitStack() as p1ctx:
                p1pools = (
                    p1ctx.enter_context(tc.tile_pool(name="hblk", bufs=6)),
                    p1ctx.enter_context(tc.tile_pool(name="wch", bufs=6)),
                    p1ctx.enter_context(tc.tile_pool(name="cs", bufs=5)),
                    p1ctx.enter_context(tc.tile_pool(name="rtmp", bufs=3)),
                    p1ctx.enter_context(tc.tile_pool(name="stage", bufs=4)),
                )
                Filler([p1_stream(range(4), p1pools)]).drain()
                fb = Filler([p1_stream(range(4, NTB), p1pools)])
                tiles = prefetch_pair(0, 0)
                fb.pull(12000)
                for hh in range(HC):
                    nxt = prefetch_pair(0, hh + 1) if hh < HC - 1 else None
                    run_pair(0, hh, tiles, fb)
                    tiles = nxt
                fb.drain()

            with ExitStack() as p3ctx:
                p3pools = (
                    p3ctx.enter_context(tc.tile_pool(name="wop", bufs=3)),
                    p3ctx.enter_context(tc.tile_pool(name="ost", bufs=4)),
                )
                tiles = prefetch_pair(1, 0)
                fc = Filler([p3_stream(range(4), p3pools)])
                fc.pull(12000)
                for hh in range(HC):
                    nxt = prefetch_pair(1, hh + 1) if hh < HC - 1 else None
                    run_pair(1, hh, tiles, fc)
                    tiles = nxt
                fc.drain()
                Filler([p3_stream(range(4, NTB), p3pools)]).drain()

    nc.compile()
    return nc


_NC_CACHE = {}


def get_nc():
    if "nc" not in _NC_CACHE:
        _NC_CACHE["nc"] = build_nc()
    return _NC_CACHE["nc"]


def prep_in_maps(positions, hidden_states, W_qkv, W_o):
    """Host-side sharding + layout prep. Returns per-core input maps."""
    bf16 = ml_dtypes.bfloat16
    hid = np.asarray(hidden_states, np.float32).reshape(TOK, HIDDEN)
    # hT[T, i, p, kc, t] = hid[T*512+t, i*1024+kc*128+p]
    hT = np.ascontiguousarray(
        hid.reshape(NTB, TBLK, 4, 8, 128).transpose(0, 2, 4, 3, 1)
    ).reshape(NTB, 4, 128, 8 * TBLK).astype(bf16)

    pos = np.asarray(positions).reshape(TOK).astype(np.float32)
    half = D // 2
    inv = ROPE_BASE ** (-np.arange(half, dtype=np.float32) / half)
    ang = pos[:, None] * inv[None, :]                      # [TOK, 64]
    cos = np.cos(ang)
    sin = np.sin(ang)
    cos128 = np.concatenate([cos, cos], axis=1)            # [TOK, 128]
    sin128 = np.concatenate([-sin, sin], axis=1)
    csn = np.empty((TOK, 2, FH), np.float32)
    csn[:, 0, :] = np.tile(cos128, HC)
    csn[:, 1, :] = np.tile(sin128, HC)
    csn = csn.astype(bf16)

    kk = np.arange(128)[:, None]
    qq = np.arange(QT)[None, :]
    msk = np.stack([(qq >= kk + o * 128) for o in range(4)], axis=1)
    msk = msk.reshape(128, 2, 2, QT).astype(bf16)           # [128, 2, 2, 512]

    Wq = np.asarray(W_qkv, np.float32)
    Wo = np.asarray(W_o, np.float32)
    in_maps = []
    for c in range(N_CORES):
        wc = np.concatenate(
            [Wq[:, q0 * HIDDEN + c * FH: q0 * HIDDEN + (c + 1) * FH]
             for q0 in range(3)], axis=1)                   # [4096, 1536]
        # w[j3, i, p, kc, f] = wc[i*1024+kc*128+p, j3*512+f]
        wcp = np.ascontiguousarray(
            wc.reshape(4, 8, 128, 3, FH).transpose(3, 0, 2, 1, 4)
        ).reshape(3, 4, 128, 8 * FH).astype(bf16)
        woc = Wo[c * FH:(c + 1) * FH, :]                    # [512, 4096]
        # wo[ob, p, kc, o] = woc[kc*128+p, ob*128+o]
        wop = np.ascontiguousarray(
            woc.reshape(HC, 128, NOB, 128).transpose(2, 1, 0, 3)
        ).astype(bf16)
        in_maps.append({"hT": hT, "w": wcp, "wo": wop, "csn": csn, "msk": msk})
    return in_maps


def combine_outputs(outps):
    """Sum per-core o_partial^T [HIDDEN, TOK] and untranspose."""
    out = outps[0].astype(np.float64)
    for o in outps[1:]:
        out += o
    return np.ascontiguousarray(out.T).astype(np.float32).reshape(B, S, HIDDEN)


def kernel(positions, hidden_states, W_qkv, W_o):
    nc = get_nc()
    in_maps = prep_in_maps(positions, hidden_states, W_qkv, W_o)
    res = run_bass_kernel_spmd(nc, in_maps, list(range(N_CORES)))
    return combine_outputs([res.results[c]["outp"] for c in range(N_CORES)])
